# revision 8
# baseline (speedup 1.0000x reference)
"""NT-Xent children-fmaps loss on 8 Trainium2 NeuronCores.

Math: all needed cosine similarities reduce to per-child-pair Gram blocks
  G_bank[f, (j,cb), (ca,i)] = <bank[f, j, cb, :], fmaps_i[f, i, ca, :]>   (D=512 dots)
The gathered-negative dot products are then one-hot masked sums:
  dot_bank[f,i,j] = sum_ca G_bank[f, (j, idx[f,i,j,ca]), (ca,i)]
Device work (per core, bank-rows j sharded 8-way):
  PE Gram matmuls (bf16)  ->  DVE one-hot mask multiply  ->  PE delta-weight
  accumulating matmuls (sums over ca via PSUM accumulation and over cb via the
  contraction) -> tiny [3,8,128] result DMA'd out.
Norms, cosine scaling, masking and logsumexp run on host in float64 over the
gathered [3,64,64] dot tensors (0.01% of the FLOPs).
"""

import sys

sys.path.insert(0, "/opt/trn_rl_repo")

import ml_dtypes
import numpy as np

F, B, C, D = 3, 64, 16, 512
M_CORES = 8
JS = B // M_CORES  # 8 bank rows per core
KC = D // 128  # 4 contraction chunks
TEMPERATURE = 0.1
EPS = 1e-8
NEG_INF = -1e30

_compiled = {}


def _build_bass():
    import concourse.bacc as bacc
    import concourse.mybir as mybir
    import concourse.tile as tile

    dt = mybir.dt
    nc = bacc.Bacc()

    fit_d = nc.declare_dram_parameter("fit", [F, KC, 128, 1024], dt.bfloat16, isOutput=False)
    fjt_d = nc.declare_dram_parameter("fjt", [F, KC, 128, 128], dt.bfloat16, isOutput=False)
    fkt_d = nc.declare_dram_parameter("fkt", [F, KC, 128, 128], dt.bfloat16, isOutput=False)
    mj_d = nc.declare_dram_parameter("mj", [F, 128, 1024], dt.bfloat16, isOutput=False)
    mk_d = nc.declare_dram_parameter("mk", [F, 128, 1024], dt.bfloat16, isOutput=False)
    w2_d = nc.declare_dram_parameter("w2", [128, 8], dt.bfloat16, isOutput=False)
    out_d = nc.declare_dram_parameter("dots", [F, 8, 128], dt.float32, isOutput=True)

    with tile.TileContext(nc) as tc:
        with (
            tc.tile_pool(name="consts", bufs=1) as consts,
            tc.tile_pool(name="fit", bufs=2) as fit_pool,
            tc.tile_pool(name="banks", bufs=2) as bank_pool,
            tc.tile_pool(name="masks", bufs=2) as mask_pool,
            tc.tile_pool(name="gm", bufs=2) as gm_pool,
            tc.tile_pool(name="gt", bufs=2, space="PSUM") as gt_pool,
            tc.tile_pool(name="dots", bufs=2, space="PSUM") as dots_pool,
        ):
            w2_sb = consts.tile([128, 8], dt.bfloat16)
            nc.sync.dma_start(out=w2_sb, in_=w2_d[:, :])

            for f in range(F):
                fit_sb = fit_pool.tile([128, KC, 1024], dt.bfloat16, tag="fit")
                nc.sync.dma_start(
                    out=fit_sb, in_=fit_d[f].rearrange("k p c -> p k c")
                )
                fjt_sb = bank_pool.tile([128, KC, 128], dt.bfloat16, tag="fjt")
                nc.sync.dma_start(
                    out=fjt_sb, in_=fjt_d[f].rearrange("k p c -> p k c")
                )
                fkt_sb = bank_pool.tile([128, KC, 128], dt.bfloat16, tag="fkt")
                nc.sync.dma_start(
                    out=fkt_sb, in_=fkt_d[f].rearrange("k p c -> p k c")
                )
                mj_sb = mask_pool.tile([128, 1024], dt.bfloat16, tag="mj")
                nc.sync.dma_start(out=mj_sb, in_=mj_d[f])
                mk_sb = mask_pool.tile([128, 1024], dt.bfloat16, tag="mk")
                nc.sync.dma_start(out=mk_sb, in_=mk_d[f])

                dots_ps = dots_pool.tile([8, 128], dt.float32, tag="dots")
                for bi, (bank_sb, m_sb) in enumerate(
                    ((fjt_sb, mj_sb), (fkt_sb, mk_sb))
                ):
                    gt_ps = gt_pool.tile([128, 1024], dt.float32, tag="gt")
                    for h in range(2):
                        for kc in range(KC):
                            nc.tensor.matmul(
                                gt_ps[:, h * 512 : (h + 1) * 512],
                                bank_sb[:, kc, :],
                                fit_sb[:, kc, h * 512 : (h + 1) * 512],
                                start=(kc == 0),
                                stop=(kc == KC - 1),
                            )
                    gm_sb = gm_pool.tile([128, 1024], dt.bfloat16, tag="gm")
                    for h in range(2):
                        sl = slice(h * 512, (h + 1) * 512)
                        nc.vector.tensor_mul(gm_sb[:, sl], gt_ps[:, sl], m_sb[:, sl])
                    for ca in range(C):
                        nc.tensor.matmul(
                            dots_ps[:, bi * 64 : (bi + 1) * 64],
                            w2_sb,
                            gm_sb[:, ca * 64 : (ca + 1) * 64],
                            start=(ca == 0),
                            stop=(ca == C - 1),
                        )
                dots_sb = gm_pool.tile([8, 128], dt.float32, tag="dots_sb")
                nc.vector.tensor_copy(dots_sb, dots_ps)
                nc.sync.dma_start(out=out_d[f], in_=dots_sb)

    nc.finalize()
    return nc


def _get_nc():
    if "nc" not in _compiled:
        _compiled["nc"] = _build_bass()
    return _compiled["nc"]


def kernel(fmaps_i, fmaps_j, partnet_ids, neg_idx_j, neg_idx_k):
    from concourse.bass_utils import run_bass_kernel_spmd

    fi = np.asarray(fmaps_i, dtype=np.float32)
    fj = np.asarray(fmaps_j, dtype=np.float32)
    ids = np.asarray(partnet_ids)
    ij = np.asarray(neg_idx_j)
    ik = np.asarray(neg_idx_k)

    bf16 = ml_dtypes.bfloat16
    # fit[f, kc, p, ca*64+i] = fi[f, i, ca, kc*128+p]
    fit = np.ascontiguousarray(
        fi.transpose(0, 3, 2, 1).reshape(F, KC, 128, C * B)
    ).astype(bf16)
    w2 = np.zeros((128, 8), dtype=bf16)
    for jl in range(8):
        w2[jl * 16 : (jl + 1) * 16, jl] = 1

    cb = np.arange(C, dtype=ij.dtype)
    in_maps = []
    for m in range(M_CORES):
        sl = slice(m * JS, (m + 1) * JS)
        # bank cols = jl*16+cb; bankT[f, kc, p, col] = bank[f, m*8+jl, cb, kc*128+p]
        fjt = np.ascontiguousarray(
            fj[:, sl].transpose(0, 3, 1, 2).reshape(F, KC, 128, 128)
        ).astype(bf16)
        fkt = np.ascontiguousarray(
            fi[:, sl].transpose(0, 3, 1, 2).reshape(F, KC, 128, 128)
        ).astype(bf16)
        # M[f, jl*16+cb, ca*64+i] = (idx[f, i, m*8+jl, ca] == cb)
        mj = (
            (ij[:, :, sl, :].transpose(0, 2, 3, 1)[:, :, None, :, :] == cb[None, None, :, None, None])
            .reshape(F, 128, C * B)
            .astype(bf16)
        )
        mk = (
            (ik[:, :, sl, :].transpose(0, 2, 3, 1)[:, :, None, :, :] == cb[None, None, :, None, None])
            .reshape(F, 128, C * B)
            .astype(bf16)
        )
        in_maps.append({"fit": fit, "fjt": fjt, "fkt": fkt, "mj": mj, "mk": mk, "w2": w2})

    nc = _get_nc()
    global last_in_maps
    last_in_maps = in_maps
    res = run_bass_kernel_spmd(nc, in_maps, core_ids=list(range(M_CORES)))

    # device dots: res[m]["dots"][f, jl, b*64+i] = dot for (f, i, j=m*8+jl, bank b)
    dot_j = np.empty((F, B, B), dtype=np.float64)
    dot_k = np.empty((F, B, B), dtype=np.float64)
    for m in range(M_CORES):
        d = np.asarray(res.results[m]["dots"], dtype=np.float64)  # [3, 8, 128]
        dot_j[:, :, m * JS : (m + 1) * JS] = d[:, :, 0:64].transpose(0, 2, 1)
        dot_k[:, :, m * JS : (m + 1) * JS] = d[:, :, 64:128].transpose(0, 2, 1)

    # host: norms, cosine scaling, mask, logsumexp (float64)
    fi64 = fi.astype(np.float64)
    fj64 = fj.astype(np.float64)
    Ni = (fi64**2).sum(-1)  # [F,B,C] child squared norms of fmaps_i
    Nj = (fj64**2).sum(-1)
    asq = Ni.sum(-1)  # [F,B] anchor squared norms
    psq = Nj.sum(-1)  # [F,B] positive squared norms

    gj = np.take_along_axis(
        np.broadcast_to(Nj[:, None, :, :], (F, B, B, C)), ij.astype(np.int64), axis=3
    ).sum(-1)  # [F,B,B] gathered-negative squared norms (bank j)
    gk = np.take_along_axis(
        np.broadcast_to(Ni[:, None, :, :], (F, B, B, C)), ik.astype(np.int64), axis=3
    ).sum(-1)

    na = np.maximum(np.sqrt(asq), EPS)  # [F,B]
    npos = np.maximum(np.sqrt(psq), EPS)
    nbj = np.maximum(np.sqrt(gj), EPS)  # [F,B,B]
    nbk = np.maximum(np.sqrt(gk), EPS)

    pos_dot = np.einsum("fbcd,fbcd->fb", fi64, fj64)
    pos_sim = pos_dot / (na * npos) / TEMPERATURE  # [F,B]

    sim_j = dot_j / (na[:, :, None] * nbj) / TEMPERATURE
    sim_k = dot_k / (na[:, :, None] * nbk) / TEMPERATURE

    valid = (np.arange(B)[:, None] != np.arange(B)[None, :]) & (
        ids[:, None] != ids[None, :]
    )
    sim_j = np.where(valid[None], sim_j, NEG_INF)
    sim_k = np.where(valid[None], sim_k, NEG_INF)

    logits = np.concatenate([pos_sim[..., None], sim_j, sim_k], axis=-1)  # [F,B,1+2B]
    mx = logits.max(-1)
    lse = np.log(np.exp(logits - mx[..., None]).sum(-1)) + mx
    loss = (lse - pos_sim).sum() / (2 * B)
    return np.float32(loss)
